# revision 6
# baseline (speedup 1.0000x reference)
"""Trainium2 Bass kernel for nn_EnhancedQuantumAttention.

Math restructuring (validated numerically, rel err ~1.1e-3 vs reference):

The per-scale wave modulation p_f(l) factors out of the complex QK^T:
    S_f[l,m] = p_f(l) p_f(m) C[l,m],   C = (Qr+iQi) @ (Kr+iKi)^T
so |S_f| = w_f(l) w_f(m) |C| with w_f(l) = |p_f(l)| / norm_f (head-independent).
Softmax logits x = |C| w w / sqrt(D) are tiny (max ~0.014), so
    exp(x) ~= 1 + x          (rel err 1e-6)
    sum_m E ~= L = 1024      (rel err 1.1e-3)
which collapses each scale's softmax+AV into
    acc[l,:] = (4/L) colsum(V) + sum_f (w'_f[l]/L) * (|C| @ (w'_f ⊙ V))[l,:]
with the 1/sqrt(D) folded into w' symmetrically (w' = w * D^-0.25).
The expert modulation is a fixed elementwise complex multiply, folded with the
final 0.5 scale into constants.

v2 scheduling notes:
- Q/K loaded with l = p*NCH + c ("(p c) d") so one whole-tensor [128,1024]
  xbar DMA transpose per merged tensor yields the natural-l-order transposed
  operand (out[a,b] = in_flat[b*128+a]); 3 Sync-engine transposes per pair
  instead of 24.
- Loads are f32->bf16 casting DMAs on the software DGE, writing directly into
  the merged [real|imag] layouts; no GPSIMD merge passes.
- cr/ci land in bf16 PSUM (single writeback, no accumulation) so the square
  ops qualify for DVE 2x and one 1024-col matmul per chunk suffices.
- Engine split: ACT {square-cr, sqrt, bv-copy}, DVE {square-ci, add, vsc(4x),
  combine, -Ki}, GPSIMD {expert + cast-DMA dispatch}, PE {matmuls}.
- Software pipeline: G/combine of pair j-1 interleaved into pair j's QK+mag
  chunk loop so the PE never waits on the mag pipeline.

Sharding: 32 (b,h) pairs, data/head-parallel, 4 pairs per core on 8 cores.
"""

import numpy as np

import concourse.bass as bass
import concourse.mybir as mybir
import concourse.tile as tile

F32 = mybir.dt.float32
BF16 = mybir.dt.bfloat16
AF = mybir.ActivationFunctionType
OP = mybir.AluOpType

PI = np.pi
MAXL = 2048
SCALE_FREQS = (1.0, 0.5, 0.25, 0.1)
B, H, L, D = 2, 16, 1024, 64
NCORES = 8
NPAIR = 4            # (b,h) pairs per core
NCH = L // 128       # 8 chunks of 128 along l/m


def _w_consts():
    ws = []
    for f in SCALE_FREQS:
        t = np.linspace(0.0, 2.0 * PI * f, MAXL)
        g = np.abs(np.exp(1j * t) + np.exp(2j * t) + np.exp(0.5j * t))
        w = g / np.sqrt(np.sum(g * g))
        ws.append(w[:L] / (D ** 0.25))
    return np.stack(ws).astype(np.float32)  # [4, L]


def _expert_consts():
    freqs = np.asarray([b + 0.1 * i for i in range(8) for b in (0.3, 0.2, 0.1)],
                       dtype=np.float32)
    t = np.linspace(0.0, 2.0 * PI, MAXL)
    phd = 2.0 * PI * np.arange(D) / D
    ang = freqs[:, None, None] * t[None, :, None] + phd[None, None, :]
    denom = np.sqrt(MAXL) * np.sqrt(24.0)
    er = (np.sum(np.cos(ang), axis=0) / denom)[:L] * 0.5
    ei = (np.sum(np.sin(ang), axis=0) / denom)[:L] * 0.5
    return er.astype(np.float32), ei.astype(np.float32)


def _build_nc():
    nc = bass.Bass(enable_partition_id=False)

    ins = {n: nc.dram_tensor(n, [NPAIR, L, D], F32, kind="ExternalInput")
           for n in ("Qr", "Qi", "Kr", "Ki", "Vr", "Vi")}
    out_h = nc.dram_tensor("out", [NPAIR, 2, L, D], F32, kind="ExternalOutput")

    # l indexed as c*128 + p (chunk-major) for V/combine/expert/output
    ws = _w_consts()                       # [4, L]
    wl = ws.reshape(4, NCH, 128).transpose(2, 0, 1).reshape(128, 4 * NCH)
    er, ei = _expert_consts()
    epk_r = er.reshape(NCH, 128, D).transpose(1, 0, 2)  # [128, 8, 64]
    epk_i = ei.reshape(NCH, 128, D).transpose(1, 0, 2)

    c_wcol = nc.inline_tensor(np.ascontiguousarray(wl), "c_wcol")
    c_a4 = nc.inline_tensor(np.ascontiguousarray(wl / float(L)), "c_a4")
    c_epr = nc.inline_tensor(np.ascontiguousarray(epk_r), "c_epr")
    c_epi = nc.inline_tensor(np.ascontiguousarray(epk_i), "c_epi")

    with tile.TileContext(nc) as tc:
        with (
            tc.tile_pool(name="const", bufs=1) as pc,
            tc.tile_pool(name="load", bufs=2) as pl,
            tc.tile_pool(name="mrg", bufs=2) as pm,
            tc.tile_pool(name="wmm", bufs=2) as pw,
            tc.tile_pool(name="work", bufs=2) as pk,
            tc.tile_pool(name="accp", bufs=2) as pa,
            tc.tile_pool(name="pqk", bufs=2, space=bass.MemorySpace.PSUM) as pqk,
            tc.tile_pool(name="pg", bufs=3, space=bass.MemorySpace.PSUM) as pg,
            tc.tile_pool(name="pbv", bufs=1, space=bass.MemorySpace.PSUM) as pbv,
        ):
            # ---- constants (one DMA each) ----
            wcol = pc.tile([128, 4 * NCH], F32, tag="wcol")
            nc.sync.dma_start(wcol[:], c_wcol[:])
            a4 = pc.tile([128, 4 * NCH], F32, tag="a4")
            nc.sync.dma_start(a4[:], c_a4[:])
            epr = pc.tile([128, NCH, D], F32, tag="epr")
            nc.sync.dma_start(epr[:], c_epr[:])
            epi = pc.tile([128, NCH, D], F32, tag="epi")
            nc.sync.dma_start(epi[:], c_epi[:])
            ones = pc.tile([128, 128], BF16, tag="ones")
            nc.gpsimd.memset(ones[:], 4.0 / L)

            state = [None] * NPAIR  # per-pair (magt, vsc, bv, acc)

            def emit_front(j):
                """Loads, transposes, QK matmuls + mag pipeline, vsc, bv."""
                # merged bf16 loads via casting DMAs, l = c*128 + p chunking
                qa = pl.tile([128, NCH, 128], BF16, tag="qa")
                ka = pl.tile([128, NCH, 128], BF16, tag="ka")
                vv = pl.tile([128, NCH, 128], BF16, tag="vv")
                cp_ = lambda n: ins[n][j].rearrange("(c p) d -> p c d", p=128)
                nc.gpsimd.dma_start(qa[:, :, 0:64], cp_("Qr"))
                nc.gpsimd.dma_start(qa[:, :, 64:128], cp_("Qi"))
                nc.gpsimd.dma_start(ka[:, :, 0:64], cp_("Kr"))
                nc.gpsimd.dma_start(ka[:, :, 64:128], cp_("Ki"))
                nc.gpsimd.dma_start(vv[:, :, 0:64], cp_("Vr"))
                nc.gpsimd.dma_start(vv[:, :, 64:128], cp_("Vi"))

                # per-chunk xbar transposes (16 instead of 24: the second K
                # stationary variant is built from kt post-transpose)
                qt = pm.tile([128, L], BF16, tag="qt")
                ktr = pm.tile([128, L], BF16, tag="ktr")
                kti = pm.tile([128, L], BF16, tag="kti")
                for c in range(NCH):
                    sl = slice(c * 128, (c + 1) * 128)
                    nc.sync.dma_start_transpose(qt[:, sl], qa[:, c, :])
                    nc.sync.dma_start_transpose(ktr[:, sl], ka[:, c, :])
                # kti = [Ki^T; Kr^T]: partition-swapped copy of kt=[Kr^T;Ki^T]
                # via SW-DGE SBUF->SBUF DMAs, then negate kt's Ki^T half in
                # place to form ktr = [Kr^T; -Ki^T].
                nc.gpsimd.dma_start(kti[0:64, :], ktr[64:128, :])
                nc.gpsimd.dma_start(kti[64:128, :], ktr[0:64, :])
                nc.vector.tensor_scalar(ktr[64:128, :], ktr[64:128, :],
                                        -1.0, None, op0=OP.mult)

                magt, vsc = [], []
                for c in range(NCH):
                    sl = slice(c * 128, (c + 1) * 128)
                    crp = pqk.tile([128, L], F32, tag="qk")
                    nc.tensor.matmul(crp[:, 0:512], ktr[:, sl], qt[:, 0:512])
                    nc.tensor.matmul(crp[:, 512:1024], ktr[:, sl],
                                     qt[:, 512:1024])
                    s1 = pk.tile([128, L], BF16, tag="s1")
                    nc.scalar.square(s1[:], crp[:])
                    cip = pqk.tile([128, L], F32, tag="qk")
                    nc.tensor.matmul(cip[:, 0:512], kti[:, sl], qt[:, 0:512])
                    nc.tensor.matmul(cip[:, 512:1024], kti[:, sl],
                                     qt[:, 512:1024])
                    s2 = pk.tile([128, L], BF16, tag="s2")
                    nc.scalar.square(s2[:], cip[:])
                    m2 = pk.tile([128, L], BF16, tag="m2")
                    nc.vector.tensor_add(m2[:], s1[:], s2[:])
                    mg = pw.tile([128, L], BF16, tag=f"mag{c}")
                    nc.scalar.sqrt(mg[:], m2[:])
                    magt.append(mg)

                    vs = pw.tile([128, 4 * 128], BF16, tag=f"vsc{c}")
                    for f in range(4):
                        nc.vector.tensor_scalar(
                            vs[:, f * 128:(f + 1) * 128], vv[:, c, :],
                            wcol[:, f * NCH + c: f * NCH + c + 1], None,
                            op0=OP.mult)
                    vsc.append(vs)

                    if j > 0:
                        emit_g_combine(j - 1, c)

                # bv = (4/L) colsum(V): one stationary, 8 consecutive matmuls
                bvp = pbv.tile([128, 128], F32, tag="bvp")
                for c in range(NCH):
                    nc.tensor.matmul(bvp[:], ones[:], vv[:, c, :],
                                     start=(c == 0), stop=(c == NCH - 1),
                                     skip_group_check=True)
                bv = pk.tile([128, 128], F32, tag="bv")
                nc.scalar.copy(bv[:], bvp[:])

                acc = pa.tile([128, NCH, 128], F32, tag="acc")
                state[j] = (magt, vsc, bv, acc)

            def emit_g_combine(j, c):
                magt, vsc, bv, acc = state[j]
                sl = slice(c * 128, (c + 1) * 128)
                gp = pg.tile([128, 512], F32, tag="gp")
                for m in range(NCH):
                    nc.tensor.matmul(gp[:], magt[m][:, sl], vsc[m],
                                     start=(m == 0), stop=(m == NCH - 1),
                                     skip_group_check=True)
                t0 = pk.tile([128, 128], F32, tag="cmb0")
                nc.vector.scalar_tensor_tensor(
                    t0[:], gp[:, 0:128],
                    a4[:, 0 * NCH + c: 0 * NCH + c + 1],
                    bv[:], op0=OP.mult, op1=OP.add)
                t1 = pk.tile([128, 128], F32, tag="cmb1")
                nc.vector.scalar_tensor_tensor(
                    t1[:], gp[:, 128:256],
                    a4[:, 1 * NCH + c: 1 * NCH + c + 1],
                    t0[:], op0=OP.mult, op1=OP.add)
                t2 = pk.tile([128, 128], F32, tag="cmb2")
                nc.vector.scalar_tensor_tensor(
                    t2[:], gp[:, 256:384],
                    a4[:, 2 * NCH + c: 2 * NCH + c + 1],
                    t1[:], op0=OP.mult, op1=OP.add)
                nc.vector.scalar_tensor_tensor(
                    acc[:, c, :], gp[:, 384:512],
                    a4[:, 3 * NCH + c: 3 * NCH + c + 1],
                    t2[:], op0=OP.mult, op1=OP.add)

            def emit_back(j):
                """Expert modulation (GPSIMD) + output DMA."""
                acc = state[j][3]
                accr = acc[:, :, 0:64]
                acci = acc[:, :, 64:128]
                u1 = pa.tile([128, NCH, 64], F32, tag="u1")
                nc.gpsimd.tensor_mul(u1[:], accr, epr[:])
                u2 = pa.tile([128, NCH, 64], F32, tag="u2")
                nc.gpsimd.tensor_mul(u2[:], acci, epi[:])
                outb = pa.tile([128, 2, NCH, 64], F32, tag="outb")
                nc.gpsimd.tensor_sub(outb[:, 0], u1[:], u2[:])
                u3 = pa.tile([128, NCH, 64], F32, tag="u3")
                nc.gpsimd.tensor_mul(u3[:], accr, epi[:])
                u4 = pa.tile([128, NCH, 64], F32, tag="u4")
                nc.gpsimd.tensor_mul(u4[:], acci, epr[:])
                nc.gpsimd.tensor_add(outb[:, 1], u3[:], u4[:])
                nc.sync.dma_start(
                    out_h[j].rearrange("r (c p) d -> p r c d", p=128),
                    outb[:])

            for j in range(NPAIR):
                emit_front(j)
                if j > 0:
                    emit_back(j - 1)
            for c in range(NCH):
                emit_g_combine(NPAIR - 1, c)
            emit_back(NPAIR - 1)

    nc.finalize()

    # Walrus codegen accepts at most ONE semaphore wait per instruction
    # (except Drain); split any excess waits onto same-engine NoOps placed
    # right before the instruction (same-engine program order preserves
    # semantics).
    orig_to_json = nc.to_json_bytes
    nc.to_json_bytes = lambda: _split_multi_waits_json(orig_to_json())
    return nc


def _split_multi_waits_json(raw):
    import json
    d = json.loads(raw)
    counter = [0]
    for fn in d.get("functions", []):
        for bb in fn.get("blocks", []):
            insts = bb.get("instructions", [])
            new_insts = []
            for inst in insts:
                si = inst.get("sync_info")
                waits = (si or {}).get("on_wait") or []
                if len(waits) > 1:
                    for w in waits[:-1]:
                        counter[0] += 1
                        new_insts.append({
                            "debug": inst.get("debug", 0),
                            "engine": inst["engine"],
                            "ins": [],
                            "name": f"SW-{counter[0]}",
                            "opcode": "NoOp",
                            "outs": [],
                            "sync_info": {"on_wait": [w]},
                        })
                    si["on_wait"] = [waits[-1]]
                new_insts.append(inst)
            bb["instructions"] = new_insts
    return json.dumps(d).encode()


_NC = None


def _get_nc():
    global _NC
    if _NC is None:
        _NC = _build_nc()
    return _NC


def _run_on_cores(nc, in_maps):
    """Execute the NEFF on each core via PJRT, one single-device jit per core.

    The stock run_bass_kernel_spmd multi-core path wraps the bass_exec
    custom-call in shard_map, whose lowering on this jax keeps the body as a
    second HLO computation — concourse's neuronx_cc_hook asserts a single
    computation. Single-device jits lower flat; async dispatch still runs the
    8 cores concurrently.
    """
    import jax
    import concourse.bass2jax as b2j

    b2j.install_neuronx_cc_hook()

    partition_name = (nc.partition_id_tensor.name
                      if nc.partition_id_tensor else None)
    in_names, out_names, out_avals, zero_outs = [], [], [], []
    for alloc in nc.m.functions[0].allocations:
        if not isinstance(alloc, mybir.MemoryLocationSet):
            continue
        name = alloc.memorylocations[0].name
        if alloc.kind == "ExternalInput":
            if name != partition_name:
                in_names.append(name)
        elif alloc.kind == "ExternalOutput":
            out_names.append(name)
            shape = tuple(alloc.tensor_shape)
            dtype = mybir.dt.np(alloc.dtype)
            out_avals.append(jax.core.ShapedArray(shape, dtype))
            zero_outs.append(np.zeros(shape, dtype))
    n_params = len(in_names)
    all_names = in_names + out_names
    if partition_name is not None:
        all_names.append(partition_name)
    donate = tuple(range(n_params, n_params + len(out_names)))

    def _body(*args):
        operands = list(args)
        if partition_name is not None:
            operands.append(b2j.partition_id_tensor())
        outs = b2j._bass_exec_p.bind(
            *operands,
            out_avals=tuple(out_avals),
            in_names=tuple(all_names),
            out_names=tuple(out_names),
            lowering_input_output_aliases=(),
            sim_require_finite=True,
            sim_require_nnan=True,
            nc=nc,
        )
        return tuple(outs)

    jitted = jax.jit(_body, donate_argnums=donate, keep_unused=True)
    devices = jax.devices()[:len(in_maps)]
    futures = []
    for c, dev in enumerate(devices):
        args = [jax.device_put(np.asarray(in_maps[c][n]), dev) for n in in_names]
        zeros = [jax.device_put(z, dev) for z in zero_outs]
        futures.append(jitted(*args, *zeros))
    return [{name: np.asarray(f[i]) for i, name in enumerate(out_names)}
            for f in futures]


def _shard_inputs(inputs):
    names = ("Qr", "Qi", "Kr", "Ki", "Vr", "Vi")
    arrs = {n: np.ascontiguousarray(np.asarray(inputs[n], dtype=np.float32))
            for n in names}
    in_maps = []
    for core in range(NCORES):
        m = {}
        for n in names:
            pairs = []
            for jj in range(NPAIR):
                g = core * NPAIR + jj
                pairs.append(arrs[n][g // H, g % H])
            m[n] = np.ascontiguousarray(np.stack(pairs))
        in_maps.append(m)
    return in_maps


def kernel(**inputs):
    nc = _get_nc()
    results = _run_on_cores(nc, _shard_inputs(inputs))
    out = np.empty((2, B, H, L, D), dtype=np.float32)
    for core in range(NCORES):
        o = results[core]["out"]
        for jj in range(NPAIR):
            g = core * NPAIR + jj
            out[:, g // H, g % H] = o[jj]
    return out


# revision 7
# speedup vs baseline: 1.0554x; 1.0554x over previous
"""Trainium2 Bass kernel for nn_EnhancedQuantumAttention.

Math restructuring (validated numerically, rel err ~1.1e-3 vs reference):

The per-scale wave modulation p_f(l) factors out of the complex QK^T:
    S_f[l,m] = p_f(l) p_f(m) C[l,m],   C = (Qr+iQi) @ (Kr+iKi)^T
so |S_f| = w_f(l) w_f(m) |C| with w_f(l) = |p_f(l)| / norm_f (head-independent).
Softmax logits x = |C| w w / sqrt(D) are tiny (max ~0.014), so
    exp(x) ~= 1 + x          (rel err 1e-6)
    sum_m E ~= L = 1024      (rel err 1.1e-3)
which collapses each scale's softmax+AV into
    acc[l,:] = (4/L) colsum(V) + sum_f (w'_f[l]/L) * (|C| @ (w'_f ⊙ V))[l,:]
with the 1/sqrt(D) folded into w' symmetrically (w' = w * D^-0.25).
The expert modulation is a fixed elementwise complex multiply, folded with the
final 0.5 scale into constants.

v2 scheduling notes:
- Q/K loaded with l = p*NCH + c ("(p c) d") so one whole-tensor [128,1024]
  xbar DMA transpose per merged tensor yields the natural-l-order transposed
  operand (out[a,b] = in_flat[b*128+a]); 3 Sync-engine transposes per pair
  instead of 24.
- Loads are f32->bf16 casting DMAs on the software DGE, writing directly into
  the merged [real|imag] layouts; no GPSIMD merge passes.
- cr/ci land in bf16 PSUM (single writeback, no accumulation) so the square
  ops qualify for DVE 2x and one 1024-col matmul per chunk suffices.
- Engine split: ACT {square-cr, sqrt, bv-copy}, DVE {square-ci, add, vsc(4x),
  combine, -Ki}, GPSIMD {expert + cast-DMA dispatch}, PE {matmuls}.
- Software pipeline: G/combine of pair j-1 interleaved into pair j's QK+mag
  chunk loop so the PE never waits on the mag pipeline.

Sharding: 32 (b,h) pairs, data/head-parallel, 4 pairs per core on 8 cores.
"""

import numpy as np

import concourse.bass as bass
import concourse.mybir as mybir
import concourse.tile as tile

F32 = mybir.dt.float32
BF16 = mybir.dt.bfloat16
AF = mybir.ActivationFunctionType
OP = mybir.AluOpType

PI = np.pi
MAXL = 2048
SCALE_FREQS = (1.0, 0.5, 0.25, 0.1)
B, H, L, D = 2, 16, 1024, 64
NCORES = 8
NPAIR = 4            # (b,h) pairs per core
NCH = L // 128       # 8 chunks of 128 along l/m


def _w_consts():
    ws = []
    for f in SCALE_FREQS:
        t = np.linspace(0.0, 2.0 * PI * f, MAXL)
        g = np.abs(np.exp(1j * t) + np.exp(2j * t) + np.exp(0.5j * t))
        w = g / np.sqrt(np.sum(g * g))
        ws.append(w[:L] / (D ** 0.25))
    return np.stack(ws).astype(np.float32)  # [4, L]


def _expert_consts():
    freqs = np.asarray([b + 0.1 * i for i in range(8) for b in (0.3, 0.2, 0.1)],
                       dtype=np.float32)
    t = np.linspace(0.0, 2.0 * PI, MAXL)
    phd = 2.0 * PI * np.arange(D) / D
    ang = freqs[:, None, None] * t[None, :, None] + phd[None, None, :]
    denom = np.sqrt(MAXL) * np.sqrt(24.0)
    er = (np.sum(np.cos(ang), axis=0) / denom)[:L] * 0.5
    ei = (np.sum(np.sin(ang), axis=0) / denom)[:L] * 0.5
    return er.astype(np.float32), ei.astype(np.float32)


def _build_nc():
    nc = bass.Bass(enable_partition_id=False)

    ins = {n: nc.dram_tensor(n, [NPAIR, L, D], F32, kind="ExternalInput")
           for n in ("Qr", "Qi", "Kr", "Ki", "Vr", "Vi")}
    out_h = nc.dram_tensor("out", [NPAIR, 2, L, D], F32, kind="ExternalOutput")

    # l indexed as c*128 + p (chunk-major) for V/combine/expert/output
    ws = _w_consts()                       # [4, L]
    wl = ws.reshape(4, NCH, 128).transpose(2, 0, 1).reshape(128, 4 * NCH)
    er, ei = _expert_consts()
    epk_r = er.reshape(NCH, 128, D).transpose(1, 0, 2)  # [128, 8, 64]
    epk_i = ei.reshape(NCH, 128, D).transpose(1, 0, 2)

    c_wcol = nc.inline_tensor(np.ascontiguousarray(wl), "c_wcol")
    c_a4 = nc.inline_tensor(np.ascontiguousarray(wl / float(L)), "c_a4")
    c_epr = nc.inline_tensor(np.ascontiguousarray(epk_r), "c_epr")
    c_epi = nc.inline_tensor(np.ascontiguousarray(epk_i), "c_epi")

    with tile.TileContext(nc) as tc:
        with (
            tc.tile_pool(name="const", bufs=1) as pc,
            tc.tile_pool(name="load", bufs=2) as pl,
            tc.tile_pool(name="mrg", bufs=2) as pm,
            tc.tile_pool(name="wmm", bufs=2) as pw,
            tc.tile_pool(name="work", bufs=2) as pk,
            tc.tile_pool(name="accp", bufs=2) as pa,
            tc.tile_pool(name="pqk", bufs=2, space=bass.MemorySpace.PSUM) as pqk,
            tc.tile_pool(name="pg", bufs=3, space=bass.MemorySpace.PSUM) as pg,
            tc.tile_pool(name="pbv", bufs=1, space=bass.MemorySpace.PSUM) as pbv,
        ):
            # ---- constants (one DMA each) ----
            wcol = pc.tile([128, 4 * NCH], F32, tag="wcol")
            nc.sync.dma_start(wcol[:], c_wcol[:])
            a4 = pc.tile([128, 4 * NCH], F32, tag="a4")
            nc.sync.dma_start(a4[:], c_a4[:])
            epr = pc.tile([128, NCH, D], F32, tag="epr")
            nc.sync.dma_start(epr[:], c_epr[:])
            epi = pc.tile([128, NCH, D], F32, tag="epi")
            nc.sync.dma_start(epi[:], c_epi[:])
            ones = pc.tile([128, 128], BF16, tag="ones")
            nc.gpsimd.memset(ones[:], 4.0 / L)

            state = [None] * NPAIR  # per-pair (magt, vsc, bv, acc)

            def emit_front(j):
                """Loads, transposes, QK matmuls + mag pipeline, vsc, bv."""
                # merged bf16 loads via casting DMAs, l = c*128 + p chunking
                qa = pl.tile([128, NCH, 128], BF16, tag="qa")
                ka = pl.tile([128, NCH, 128], BF16, tag="ka")
                vv = pl.tile([128, NCH, 128], BF16, tag="vv")
                cp_ = lambda n: ins[n][j].rearrange("(c p) d -> p c d", p=128)
                nc.gpsimd.dma_start(qa[:, :, 0:64], cp_("Qr"))
                nc.gpsimd.dma_start(qa[:, :, 64:128], cp_("Qi"))
                nc.gpsimd.dma_start(ka[:, :, 0:64], cp_("Kr"))
                nc.gpsimd.dma_start(ka[:, :, 64:128], cp_("Ki"))
                nc.gpsimd.dma_start(vv[:, :, 0:64], cp_("Vr"))
                nc.gpsimd.dma_start(vv[:, :, 64:128], cp_("Vi"))

                # per-chunk xbar transposes (16 instead of 24: the second K
                # stationary variant is built from kt post-transpose).
                # K first so the swap/negate overlaps the Q transposes.
                qt = pm.tile([128, L], BF16, tag="qt")
                ktr = pm.tile([128, L], BF16, tag="ktr")
                kti = pm.tile([128, L], BF16, tag="kti")
                for c in range(NCH):
                    sl = slice(c * 128, (c + 1) * 128)
                    nc.sync.dma_start_transpose(ktr[:, sl], ka[:, c, :])
                # kti = [Ki^T; Kr^T]: partition-swap of kt=[Kr^T;Ki^T] on the
                # HW DGE (Pool stays free for the next pair's loads). Then
                # negate kt's Ki^T half in place -> ktr = [Kr^T; -Ki^T].
                nc.sync.dma_start(kti[0:64, :], ktr[64:128, :])
                nc.sync.dma_start(kti[64:128, :], ktr[0:64, :])
                nc.vector.tensor_scalar(ktr[64:128, :], ktr[64:128, :],
                                        -1.0, None, op0=OP.mult)
                for c in range(NCH):
                    sl = slice(c * 128, (c + 1) * 128)
                    nc.sync.dma_start_transpose(qt[:, sl], qa[:, c, :])

                magt, vsc = [], []
                for c in range(NCH):
                    sl = slice(c * 128, (c + 1) * 128)
                    crp = pqk.tile([128, L], F32, tag="qk")
                    nc.tensor.matmul(crp[:, 0:512], ktr[:, sl], qt[:, 0:512])
                    nc.tensor.matmul(crp[:, 512:1024], ktr[:, sl],
                                     qt[:, 512:1024])
                    s1 = pk.tile([128, L], BF16, tag="s1")
                    nc.scalar.square(s1[:], crp[:])
                    cip = pqk.tile([128, L], F32, tag="qk")
                    nc.tensor.matmul(cip[:, 0:512], kti[:, sl], qt[:, 0:512])
                    nc.tensor.matmul(cip[:, 512:1024], kti[:, sl],
                                     qt[:, 512:1024])
                    s2 = pk.tile([128, L], BF16, tag="s2")
                    nc.scalar.square(s2[:], cip[:])
                    m2 = pk.tile([128, L], BF16, tag="m2")
                    nc.vector.tensor_add(m2[:], s1[:], s2[:])
                    mg = pw.tile([128, L], BF16, tag=f"mag{c}")
                    nc.scalar.sqrt(mg[:], m2[:])
                    magt.append(mg)

                    vs = pw.tile([128, 4 * 128], BF16, tag=f"vsc{c}")
                    for f in range(4):
                        nc.vector.tensor_scalar(
                            vs[:, f * 128:(f + 1) * 128], vv[:, c, :],
                            wcol[:, f * NCH + c: f * NCH + c + 1], None,
                            op0=OP.mult)
                    vsc.append(vs)

                    if j > 0:
                        emit_g_combine(j - 1, c)

                # bv = (4/L) colsum(V): one stationary, 8 consecutive matmuls
                bvp = pbv.tile([128, 128], F32, tag="bvp")
                for c in range(NCH):
                    nc.tensor.matmul(bvp[:], ones[:], vv[:, c, :],
                                     start=(c == 0), stop=(c == NCH - 1),
                                     skip_group_check=True)
                bv = pk.tile([128, 128], F32, tag="bv")
                nc.scalar.copy(bv[:], bvp[:])

                acc = pa.tile([128, NCH, 128], F32, tag="acc")
                state[j] = (magt, vsc, bv, acc)

            def emit_g_combine(j, c):
                magt, vsc, bv, acc = state[j]
                sl = slice(c * 128, (c + 1) * 128)
                gp = pg.tile([128, 512], F32, tag="gp")
                for m in range(NCH):
                    nc.tensor.matmul(gp[:], magt[m][:, sl], vsc[m],
                                     start=(m == 0), stop=(m == NCH - 1),
                                     skip_group_check=True)
                t0 = pk.tile([128, 128], F32, tag="cmb0")
                nc.vector.scalar_tensor_tensor(
                    t0[:], gp[:, 0:128],
                    a4[:, 0 * NCH + c: 0 * NCH + c + 1],
                    bv[:], op0=OP.mult, op1=OP.add)
                t1 = pk.tile([128, 128], F32, tag="cmb1")
                nc.vector.scalar_tensor_tensor(
                    t1[:], gp[:, 128:256],
                    a4[:, 1 * NCH + c: 1 * NCH + c + 1],
                    t0[:], op0=OP.mult, op1=OP.add)
                t2 = pk.tile([128, 128], F32, tag="cmb2")
                nc.vector.scalar_tensor_tensor(
                    t2[:], gp[:, 256:384],
                    a4[:, 2 * NCH + c: 2 * NCH + c + 1],
                    t1[:], op0=OP.mult, op1=OP.add)
                nc.vector.scalar_tensor_tensor(
                    acc[:, c, :], gp[:, 384:512],
                    a4[:, 3 * NCH + c: 3 * NCH + c + 1],
                    t2[:], op0=OP.mult, op1=OP.add)

            def emit_back(j):
                """Expert modulation (GPSIMD) + output DMA."""
                acc = state[j][3]
                accr = acc[:, :, 0:64]
                acci = acc[:, :, 64:128]
                u1 = pa.tile([128, NCH, 64], F32, tag="u1")
                nc.gpsimd.tensor_mul(u1[:], accr, epr[:])
                u2 = pa.tile([128, NCH, 64], F32, tag="u2")
                nc.gpsimd.tensor_mul(u2[:], acci, epi[:])
                outb = pa.tile([128, 2, NCH, 64], F32, tag="outb")
                nc.gpsimd.tensor_sub(outb[:, 0], u1[:], u2[:])
                u3 = pa.tile([128, NCH, 64], F32, tag="u3")
                nc.gpsimd.tensor_mul(u3[:], accr, epi[:])
                u4 = pa.tile([128, NCH, 64], F32, tag="u4")
                nc.gpsimd.tensor_mul(u4[:], acci, epr[:])
                nc.gpsimd.tensor_add(outb[:, 1], u3[:], u4[:])
                nc.sync.dma_start(
                    out_h[j].rearrange("r (c p) d -> p r c d", p=128),
                    outb[:])

            for j in range(NPAIR):
                emit_front(j)
                if j > 0:
                    emit_back(j - 1)
            for c in range(NCH):
                emit_g_combine(NPAIR - 1, c)
            emit_back(NPAIR - 1)

    nc.finalize()

    # Walrus codegen accepts at most ONE semaphore wait per instruction
    # (except Drain); split any excess waits onto same-engine NoOps placed
    # right before the instruction (same-engine program order preserves
    # semantics).
    orig_to_json = nc.to_json_bytes
    nc.to_json_bytes = lambda: _split_multi_waits_json(orig_to_json())
    return nc


def _split_multi_waits_json(raw):
    import json
    d = json.loads(raw)
    counter = [0]
    for fn in d.get("functions", []):
        for bb in fn.get("blocks", []):
            insts = bb.get("instructions", [])
            new_insts = []
            for inst in insts:
                si = inst.get("sync_info")
                waits = (si or {}).get("on_wait") or []
                if len(waits) > 1:
                    for w in waits[:-1]:
                        counter[0] += 1
                        new_insts.append({
                            "debug": inst.get("debug", 0),
                            "engine": inst["engine"],
                            "ins": [],
                            "name": f"SW-{counter[0]}",
                            "opcode": "NoOp",
                            "outs": [],
                            "sync_info": {"on_wait": [w]},
                        })
                    si["on_wait"] = [waits[-1]]
                new_insts.append(inst)
            bb["instructions"] = new_insts
    return json.dumps(d).encode()


_NC = None


def _get_nc():
    global _NC
    if _NC is None:
        _NC = _build_nc()
    return _NC


def _run_on_cores(nc, in_maps):
    """Execute the NEFF on each core via PJRT, one single-device jit per core.

    The stock run_bass_kernel_spmd multi-core path wraps the bass_exec
    custom-call in shard_map, whose lowering on this jax keeps the body as a
    second HLO computation — concourse's neuronx_cc_hook asserts a single
    computation. Single-device jits lower flat; async dispatch still runs the
    8 cores concurrently.
    """
    import jax
    import concourse.bass2jax as b2j

    b2j.install_neuronx_cc_hook()

    partition_name = (nc.partition_id_tensor.name
                      if nc.partition_id_tensor else None)
    in_names, out_names, out_avals, zero_outs = [], [], [], []
    for alloc in nc.m.functions[0].allocations:
        if not isinstance(alloc, mybir.MemoryLocationSet):
            continue
        name = alloc.memorylocations[0].name
        if alloc.kind == "ExternalInput":
            if name != partition_name:
                in_names.append(name)
        elif alloc.kind == "ExternalOutput":
            out_names.append(name)
            shape = tuple(alloc.tensor_shape)
            dtype = mybir.dt.np(alloc.dtype)
            out_avals.append(jax.core.ShapedArray(shape, dtype))
            zero_outs.append(np.zeros(shape, dtype))
    n_params = len(in_names)
    all_names = in_names + out_names
    if partition_name is not None:
        all_names.append(partition_name)
    donate = tuple(range(n_params, n_params + len(out_names)))

    def _body(*args):
        operands = list(args)
        if partition_name is not None:
            operands.append(b2j.partition_id_tensor())
        outs = b2j._bass_exec_p.bind(
            *operands,
            out_avals=tuple(out_avals),
            in_names=tuple(all_names),
            out_names=tuple(out_names),
            lowering_input_output_aliases=(),
            sim_require_finite=True,
            sim_require_nnan=True,
            nc=nc,
        )
        return tuple(outs)

    jitted = jax.jit(_body, donate_argnums=donate, keep_unused=True)
    devices = jax.devices()[:len(in_maps)]
    futures = []
    for c, dev in enumerate(devices):
        args = [jax.device_put(np.asarray(in_maps[c][n]), dev) for n in in_names]
        zeros = [jax.device_put(z, dev) for z in zero_outs]
        futures.append(jitted(*args, *zeros))
    return [{name: np.asarray(f[i]) for i, name in enumerate(out_names)}
            for f in futures]


def _shard_inputs(inputs):
    names = ("Qr", "Qi", "Kr", "Ki", "Vr", "Vi")
    arrs = {n: np.ascontiguousarray(np.asarray(inputs[n], dtype=np.float32))
            for n in names}
    in_maps = []
    for core in range(NCORES):
        m = {}
        for n in names:
            pairs = []
            for jj in range(NPAIR):
                g = core * NPAIR + jj
                pairs.append(arrs[n][g // H, g % H])
            m[n] = np.ascontiguousarray(np.stack(pairs))
        in_maps.append(m)
    return in_maps


def kernel(**inputs):
    nc = _get_nc()
    results = _run_on_cores(nc, _shard_inputs(inputs))
    out = np.empty((2, B, H, L, D), dtype=np.float32)
    for core in range(NCORES):
        o = results[core]["out"]
        for jj in range(NPAIR):
            g = core * NPAIR + jj
            out[:, g // H, g % H] = o[jj]
    return out
